# revision 18
# baseline (speedup 1.0000x reference)
"""Trainium2 Bass kernel for nn_EquivariantScalar (segment_reduce).

Network (reference): two gated-equivariant blocks over N=50000 atoms with
F=128 features, then a linear head and a per-molecule (B=256) masked sum.

v3 strategy (vs the 154us baseline):
- Shard atoms across 8 cores (6250/core, padded to 6272 = 49*128).
- Host folds: block-1 mlp2's scalar half is absorbed into block-2 mlp1
  ((m1bs @ m2alo) as one matrix, bias folded into b1b), out_w absorbed
  into block-2 mlp2 (wcomb).  Removes one F x F matmul pass and the s1
  copy pass per atom.
- Narrow one-hot mask: each core only covers a ~33-molecule window, so
  the mask is (atoms, 64) per core instead of (atoms, 256); host places
  the window and re-assembles the (256,) result from the 8 partials.
- 1024-wide tiles: PSUM tiles span 2 banks; matmuls write 512-col
  halves; ACT/DVE consume 1024 wide (halves fixed instruction overhead).
- Engine balance per chunk: ACT does 4 squares + 2 sqrt + 2 silu;
  DVE does 2 squares (copy+mult, uniform bf16 for 2x mode), the gate
  bias, 3 gate-mults and ~2 norm adds; GpSimd takes the other adds.
- Stagger-group emission ({0},{1,2,3},{4,5,6}), sub-phase-major within
  each ACT-table window (squares -> adds -> sqrts; h1 matmuls -> silus
  -> per-chunk gate/vo) so no engine serializes behind another.
"""
import sys

if "/opt/trn_rl_repo" not in sys.path:
    sys.path.insert(0, "/opt/trn_rl_repo")

import numpy as np
import ml_dtypes

import concourse.bass as bass
import concourse.mybir as mybir
import concourse.tile as tile
from concourse.tile_rust import add_dep_helper as tile_rust_add_dep
from concourse.bass_utils import run_bass_kernel_spmd

F = 128
B = 256
N_NODES = 50000
N_CORES = 8
NPC = N_NODES // N_CORES          # 6250 atoms per core
PAD = 6272                        # 49 * 128
CHUNK = 1024
BF16 = mybir.dt.bfloat16
FP32 = mybir.dt.float32
AF = mybir.ActivationFunctionType
ALU = mybir.AluOpType

WNAMES = ["w2aT", "m1asT", "m1avT", "m2ahiT", "w1aT", "w2bT", "mSbT", "m1bvT"]
NBIAS = 5  # b1a, b2ahi, b1bp, bconst, zero

_CACHE = {}


def _chunks():
    out = []
    n0 = 0
    while n0 < PAD:
        w = min(CHUNK, PAD - n0)
        out.append((n0, w))
        n0 += w
    # tail (128-wide) chunk first: its tiny sv DMA lands almost
    # immediately, so compute starts ~8us earlier
    return out[-1:] + out[:-1]


def _halves(w):
    return [(0, min(512, w))] + ([(512, w - 512)] if w > 512 else [])


def _build(ncol):
    nc = bass.Bass("TRN2", debug=False)

    svT = nc.dram_tensor("svT", (F, 4, PAD), BF16, kind="ExternalInput")
    mT = nc.dram_tensor("mT", (F, PAD // F, ncol), BF16, kind="ExternalInput")
    wpk_d = nc.dram_tensor("wpk", (F, len(WNAMES), F), BF16, kind="ExternalInput")
    wc_d = nc.dram_tensor("wcombT", (F, 1), BF16, kind="ExternalInput")
    bpk_d = nc.dram_tensor("bpk", (F, NBIAS), FP32, kind="ExternalInput")
    y_d = nc.dram_tensor("y", (1, ncol), FP32, kind="ExternalOutput")

    chunks = _chunks()
    nchunks = len(chunks)
    groups = [[0], [1, 2, 3], list(range(4, nchunks))]

    last_act = [None]

    with nc.allow_low_precision(reason="bf16 intermediates intentional"):
        with tile.TileContext(nc) as tc:
            with (
                tc.tile_pool(name="wp", bufs=1) as wp,
                tc.tile_pool(name="wk", bufs=3) as wk,
                tc.tile_pool(name="ps", bufs=1, space="PSUM") as ps,
            ):
                wpk = wp.tile([F, len(WNAMES), F], BF16, name="wpk_sb")
                W = {n: wpk[:, i, :] for i, n in enumerate(WNAMES)}
                wcomb = wp.tile([F, 1], BF16, name="wcomb_sb")
                bpk = wp.tile([F, NBIAS], FP32, name="bpk_sb")
                b1a = bpk[:, 0:1]
                b2ahi = bpk[:, 1:2]
                b1bp = bpk[:, 2:3]
                bconst = bpk[:, 3:4]
                zero = bpk[:, 4:5]

                sv = wp.tile([F, 4, PAD], BF16, name="sv_sb")
                mTs = wp.tile([F, PAD // F, ncol], BF16, name="mT_sb")

                def dma_sv(cis):
                    for ci in cis:
                        n0, w = chunks[ci]
                        nc.sync.dma_start(sv[:, :, n0:n0 + w],
                                          svT[:, :, n0:n0 + w])

                # critical path first: W2a + chunk-0 inputs + biases, then
                # the rest of the weights and the next group's sv.  DMAs for
                # later groups / the mask are deferred behind an early
                # compute instruction so they don't steal bandwidth from the
                # first chunk (the sync engine otherwise issues everything
                # immediately).
                nc.sync.dma_start(wpk[:, 0:1, :], wpk_d[:, 0:1, :])
                dma_sv(groups[0])
                nc.sync.dma_start(bpk[:], bpk_d[:])
                nc.sync.dma_start(wpk[:, 1:, :], wpk_d[:, 1:, :])
                deferred = [(nc.gpsimd, wcomb[:], wc_d[:]),
                            (nc.gpsimd, mTs[:], mT[:])]

                v2n = wp.tile([F, PAD], BF16, name="v2n_sb")
                h1 = wp.tile([F, PAD], BF16, name="h1_sb")
                vo = wp.tile([F, 3, PAD], BF16, name="vo_sb")
                v2nb = wp.tile([F, PAD], BF16, name="v2nb_sb")
                hb = wp.tile([F, PAD], BF16, name="hb_sb")

                y_ps = ps.tile([1, ncol], FP32, name="y_ps", tag="y", bufs=1)

                def act(*args, **kw):
                    # Chain ACT ops in emission order so the scheduler cannot
                    # interleave activation-table sets across windows.
                    inst = nc.scalar.activation(*args, **kw)
                    if last_act[0] is not None:
                        tile_rust_add_dep(inst.ins, last_act[0], sync=False,
                                          reason="act table-set ordering")
                    last_act[0] = inst.ins
                    return inst

                st = {}

                def norm_mms_sq(ci, blk):
                    """Matmuls + squares for one chunk of a norm phase.
                    blk: 'a' = block-1 (input v), 'b' = block-2 (input vo)."""
                    n0, w = chunks[ci]
                    wname = "w2aT" if blk == "a" else "w2bT"
                    # block-2 sq tiles live from the silu window into the
                    # next sqrt window, so up to a full group of them plus
                    # the current block-1 tiles are alive at once.  Separate
                    # 2D tiles per component keep every DVE AP in 2x mode.
                    sqs = [wk.tile([F, CHUNK], BF16, name=f"sq{blk}_{ci}_{c}",
                                   tag=f"sq{c}", bufs=6) for c in range(3)]
                    for c in range(3):
                        src = (sv[:, 1 + c, :] if blk == "a" else vo[:, c, :])
                        p = ps.tile([F, CHUNK], FP32, name=f"p{blk}_{ci}_{c}",
                                    tag="pa", bufs=3)
                        for h0, hw in _halves(w):
                            nc.tensor.matmul(p[:, h0:h0 + hw], W[wname],
                                             src[:, n0 + h0:n0 + h0 + hw])
                        on_act = c < 2 or (blk == "b" and ci % 2 == 1)
                        if on_act:
                            act(sqs[c][:, :w], p[:, :w], AF.Square, bias=zero)
                        else:
                            ub = wk.tile([F, CHUNK], BF16, name=f"ub{blk}_{ci}",
                                         tag="ub", bufs=3)
                            nc.vector.tensor_copy(ub[:, :w], p[:, :w])
                            nc.vector.tensor_tensor(sqs[2][:, :w], ub[:, :w],
                                                    ub[:, :w], ALU.mult)
                    st[(blk, ci)] = sqs

                def norm_adds(ci, blk, last_in_window):
                    n0, w = chunks[ci]
                    sqs = st[(blk, ci)]
                    # t01 off the critical path can go to GpSimd; the final
                    # add feeds sqrt directly, so it stays on the fast DVE
                    eng = nc.vector if last_in_window else nc.gpsimd
                    t01 = wk.tile([F, CHUNK], BF16, name=f"t01{blk}_{ci}",
                                  tag="t01", bufs=3)
                    eng.tensor_tensor(t01[:, :w], sqs[0][:, :w], sqs[1][:, :w],
                                      ALU.add)
                    nsq = wk.tile([F, CHUNK], BF16, name=f"nsq{blk}_{ci}",
                                  tag="nsq", bufs=3)
                    nc.vector.tensor_tensor(nsq[:, :w], t01[:, :w],
                                            sqs[2][:, :w], ALU.add)
                    st[(blk, ci, "nsq")] = nsq

                def norm_sqrt(ci, blk):
                    n0, w = chunks[ci]
                    dst = v2n if blk == "a" else v2nb
                    nsq = st.pop((blk, ci, "nsq"))
                    del st[(blk, ci)]
                    act(dst[:, n0:n0 + w], nsq[:, :w], AF.Sqrt, bias=zero)

                def b1_h1mms(ci):
                    n0, w = chunks[ci]
                    p_h1 = ps.tile([F, CHUNK], FP32, name=f"ph1_{ci}",
                                   tag="pa", bufs=3)
                    for h0, hw in _halves(w):
                        nc.tensor.matmul(p_h1[:, h0:h0 + hw], W["m1asT"],
                                         sv[:, 0, n0 + h0:n0 + h0 + hw],
                                         start=True, stop=False,
                                         skip_group_check=True)
                    for h0, hw in _halves(w):
                        nc.tensor.matmul(p_h1[:, h0:h0 + hw], W["m1avT"],
                                         v2n[:, n0 + h0:n0 + h0 + hw],
                                         start=False, stop=True,
                                         skip_group_check=True)
                    st[("h1", ci)] = p_h1

                def b1_silu(ci):
                    n0, w = chunks[ci]
                    p_h1 = st.pop(("h1", ci))
                    act(h1[:, n0:n0 + w], p_h1[:, :w], AF.Silu, bias=b1a)

                def b1_gate_vo(ci):
                    n0, w = chunks[ci]
                    p_g = ps.tile([F, CHUNK], FP32, name=f"pg_{ci}",
                                  tag="pa", bufs=3)
                    for h0, hw in _halves(w):
                        nc.tensor.matmul(p_g[:, h0:h0 + hw], W["m2ahiT"],
                                         h1[:, n0 + h0:n0 + h0 + hw])
                    gate = wk.tile([F, CHUNK], BF16, name=f"gate_{ci}",
                                   tag="gate", bufs=2)
                    nc.vector.tensor_scalar_add(gate[:, :w], p_g[:, :w], b2ahi)
                    for c in range(3):
                        p_v1 = ps.tile([F, CHUNK], FP32, name=f"pv1_{ci}_{c}",
                                       tag="pa", bufs=3)
                        for h0, hw in _halves(w):
                            nc.tensor.matmul(p_v1[:, h0:h0 + hw], W["w1aT"],
                                             sv[:, 1 + c, n0 + h0:n0 + h0 + hw])
                        nc.vector.tensor_tensor(vo[:, c, n0:n0 + w],
                                                p_v1[:, :w], gate[:, :w],
                                                ALU.mult)

                def b2_hbmms(ci):
                    n0, w = chunks[ci]
                    p_hb = ps.tile([F, CHUNK], FP32, name=f"phb_{ci}",
                                   tag="pa", bufs=3)
                    for h0, hw in _halves(w):
                        nc.tensor.matmul(p_hb[:, h0:h0 + hw], W["mSbT"],
                                         h1[:, n0 + h0:n0 + h0 + hw],
                                         start=True, stop=False,
                                         skip_group_check=True)
                    for h0, hw in _halves(w):
                        nc.tensor.matmul(p_hb[:, h0:h0 + hw], W["m1bvT"],
                                         v2nb[:, n0 + h0:n0 + h0 + hw],
                                         start=False, stop=True,
                                         skip_group_check=True)
                    st[("hb", ci)] = p_hb

                def b2_silu(ci):
                    n0, w = chunks[ci]
                    p_hb = st.pop(("hb", ci))
                    act(hb[:, n0:n0 + w], p_hb[:, :w], AF.Silu, bias=b1bp)

                def tail(ci):
                    n0, w = chunks[ci]
                    nsub = w // F
                    p_sf = ps.tile([F, PAD // F], FP32, name=f"psf_{ci}",
                                   tag="sfp", bufs=1)
                    for j in range(nsub):
                        nc.tensor.matmul(p_sf[:, j:j + 1],
                                         hb[:, n0 + j * F:n0 + (j + 1) * F],
                                         wcomb[:])
                    sf = wk.tile([F, PAD // F], BF16, name=f"sf_{ci}",
                                 tag="sf", bufs=2)
                    nc.vector.tensor_scalar_add(sf[:, :nsub], p_sf[:, :nsub],
                                                bconst)
                    for j in range(nsub):
                        jabs = (n0 + j * F) // F
                        first = ci == 0 and j == 0
                        last = ci == nchunks - 1 and j == nsub - 1
                        nc.tensor.matmul(y_ps[:], sf[:, j:j + 1],
                                         mTs[:, jabs, :],
                                         start=first, stop=last,
                                         skip_group_check=True)

                def s_window(cur, prev):
                    # sqrt-set window: block-1 squares of this group (their
                    # matmuls are input-only), the pending adds, and the
                    # sqrts for this group's norm-1 + previous group's
                    # norm-2 (whose squares ran inside the silu window).
                    for ci in cur:
                        norm_mms_sq(ci, "a")
                    order = ([(ci, "b") for ci in prev]
                             + [(ci, "a") for ci in cur])
                    for i, (ci, blk) in enumerate(order):
                        norm_adds(ci, blk, last_in_window=(i >= len(order) - 2))
                    for ci, blk in order:
                        norm_sqrt(ci, blk)
                    # pre-issue next-phase matmuls (their v2n/v2nb inputs are
                    # ready chunk-by-chunk) so the PE stays busy while ACT
                    # works through this window's tail -- keeps HAM warm
                    for ci in cur:
                        b1_h1mms(ci)
                    for ci in prev:
                        b2_hbmms(ci)

                def l_window(cur, prev):
                    # silu-set window: silus, then per chunk the gate/vo
                    # path followed immediately by block-2's matmuls and
                    # squares (Square is in every table set, so it keeps ACT
                    # busy here and off the sqrt window's critical path).
                    for ci in cur:
                        b1_silu(ci)
                    for ci in prev:
                        b2_silu(ci)
                    for k in range(max(len(cur), len(prev))):
                        if k < len(cur):
                            b1_gate_vo(cur[k])
                            norm_mms_sq(cur[k], "b")
                        if k < len(prev):
                            tail(prev[k])

                prev = []
                for gi, grp in enumerate(groups):
                    if gi == 0:
                        dma_sv(groups[1])
                    s_window(grp, prev)
                    if gi == 0:
                        # release the deferred DMAs now that the first
                        # chunk's inputs have landed
                        gate_inst = last_act[0]
                        for eng, dst, src in deferred:
                            di = eng.dma_start(dst, src)
                            tile_rust_add_dep(di.ins, gate_inst, sync=True,
                                              reason="defer bulk dma")
                        for g2 in groups[2:]:
                            for ci in g2:
                                n0, w = chunks[ci]
                                di = nc.sync.dma_start(
                                    sv[:, :, n0:n0 + w], svT[:, :, n0:n0 + w])
                                tile_rust_add_dep(di.ins, gate_inst, sync=True,
                                                  reason="defer bulk dma")
                    l_window(grp, prev)
                    prev = grp
                s_window([], prev)
                l_window([], prev)

                y_sb = wk.tile([1, ncol], FP32, name="y_sb", tag="ysb")
                nc.vector.tensor_copy(y_sb[:], y_ps[:])
                nc.sync.dma_start(y_d[:], y_sb[:])

    _dedupe_ldweights(nc)
    _split_sync_waits_inline(nc, max_waits=1)
    return nc


def _dedupe_ldweights(nc):
    """Drop LDWEIGHTS whose weight AP is identical to the previous load on
    the PE stream (the stationary operand is still resident). Sync waits of
    removed loads are transplanted onto the next PE instruction."""
    f = nc.m.functions[0]
    removed = 0
    for blk in f.blocks:
        new_insts = []
        last_sig = None
        pending_waits = []
        for inst in blk.instructions:
            tn = type(inst).__name__
            if getattr(inst, "engine", None) != mybir.EngineType.PE:
                new_insts.append(inst)
                continue
            if tn == "InstLdweights":
                ap = inst.ins[0]
                sig = (ap.memref, ap.offset, str(ap.ap), str(ap.dtype),
                       str(getattr(inst, "perf_mode", None)))
                if sig == last_sig:
                    si = inst.sync_info
                    if si is not None:
                        pending_waits.extend(si.on_wait or [])
                        assert not si.on_update
                    removed += 1
                    continue
                last_sig = sig
            elif tn == "InstMatmult":
                if getattr(inst, "is_transpose", False):
                    last_sig = None
            if pending_waits:
                si = inst.sync_info
                old_w = list(si.on_wait) if si and si.on_wait else []
                old_u = list(si.on_update) if si and si.on_update else []
                inst.sync_info = mybir.SyncInfo(
                    on_wait=pending_waits + old_w, on_update=old_u)
                pending_waits = []
            new_insts.append(inst)
        assert not pending_waits
        blk.instructions[:] = new_insts
    return removed


def _split_sync_waits_inline(nc, max_waits=1):
    f = nc.m.functions[0]
    counter = [0]
    for blk in f.blocks:
        new_insts = []
        for inst in blk.instructions:
            si = getattr(inst, "sync_info", None)
            waits = list(si.on_wait) if si and si.on_wait else []
            if len(waits) > max_waits:
                head, rest = waits[:-max_waits], waits[-max_waits:]
                for i in range(0, len(head), max_waits):
                    counter[0] += 1
                    nop = mybir.InstNoOp(
                        name=f"I-wsplit-{counter[0]}",
                        engine=inst.engine,
                        ins=[],
                        outs=[],
                        sync_info=mybir.SyncInfo(
                            on_wait=head[i:i + max_waits], on_update=[]),
                    )
                    new_insts.append(nop)
                inst.sync_info = mybir.SyncInfo(on_wait=rest,
                                                on_update=list(si.on_update))
            new_insts.append(inst)
        blk.instructions[:] = new_insts


def _get_nc(ncol):
    if ncol not in _CACHE:
        _CACHE[ncol] = _build(ncol)
    return _CACHE[ncol]


def _prep_inputs(s, v, batch_mask, w1, w2, mlp_w1, mlp_b1, mlp_w2, mlp_b2,
                 out_w, out_b):
    bf16 = ml_dtypes.bfloat16
    s = np.asarray(s, np.float32)
    v = np.asarray(v, np.float32)
    batch_mask = np.asarray(batch_mask, np.float32)
    w1 = np.asarray(w1, np.float32)
    w2 = np.asarray(w2, np.float32)
    mlp_w1 = np.asarray(mlp_w1, np.float32)
    mlp_b1 = np.asarray(mlp_b1, np.float32)
    mlp_w2 = np.asarray(mlp_w2, np.float32)
    mlp_b2 = np.asarray(mlp_b2, np.float32)
    out_w = np.asarray(out_w, np.float32)
    out_b = np.asarray(out_b, np.float32)

    m1a, m1b = mlp_w1[0], mlp_w1[1]
    m2a, m2b = mlp_w2[0], mlp_w2[1]
    m1bs = m1b[:, :F]
    mSb = m1bs @ m2a[:F, :]                       # folded (F,F)
    wcomb = out_w[0] @ m2b[:F, :]                 # (F,)
    bconst = float(out_w[0] @ mlp_b2[1][:F] + out_b[0])
    b1bp = mlp_b1[1] + m1bs @ mlp_b2[0][:F]       # folded bias (F,)

    wmats = {
        "w2aT": w2[0].T, "m1asT": m1a[:, :F].T, "m1avT": m1a[:, F:].T,
        "m2ahiT": m2a[F:, :].T, "w1aT": w1[0].T, "w2bT": w2[1].T,
        "mSbT": mSb.T, "m1bvT": m1b[:, F:].T,
    }
    wpk = np.stack([wmats[n] for n in WNAMES], axis=1)  # (F, 8, F)
    bpk = np.stack([mlp_b1[0], mlp_b2[0][F:], b1bp,
                    np.full(F, bconst, np.float32),
                    np.zeros(F, np.float32)], axis=1)   # (F, 5)

    shared = {
        "wpk": np.ascontiguousarray(wpk.astype(bf16)),
        "wcombT": np.ascontiguousarray(wcomb[:, None].astype(bf16)),
        "bpk": np.ascontiguousarray(bpk.astype(np.float32)),
    }

    seg = batch_mask[:, :, 0].argmax(axis=0).astype(np.int64)  # (N,)

    # pick narrow-mask width: per-core molecule window, padded
    ncol = 64
    los = []
    for k in range(N_CORES):
        sk = seg[k * NPC:(k + 1) * NPC]
        lo = min(int(sk.min()), B - ncol)
        if int(sk.max()) - lo >= ncol:
            ncol = B  # fallback: full-width mask
        los.append(lo)
    if ncol == B:
        los = [0] * N_CORES

    in_maps = []
    for k in range(N_CORES):
        lo_atoms, hi_atoms = k * NPC, (k + 1) * NPC
        sk = np.zeros((PAD, F), np.float32)
        sk[:NPC] = s[0, lo_atoms:hi_atoms]
        vk = np.zeros((PAD, 3, F), np.float32)
        vk[:NPC] = v[0, lo_atoms:hi_atoms]
        sv = np.empty((F, 4, PAD), np.float32)
        sv[:, 0, :] = sk.T
        sv[:, 1:4, :] = vk.transpose(2, 1, 0)

        mk = np.zeros((PAD, ncol), np.float32)
        segk = seg[lo_atoms:hi_atoms] - los[k]
        mk[np.arange(NPC), segk] = 1.0

        m = dict(shared)
        m["svT"] = np.ascontiguousarray(sv.astype(bf16))
        m["mT"] = np.ascontiguousarray(
            mk.reshape(PAD // F, F, ncol).transpose(1, 0, 2).astype(bf16))
        in_maps.append(m)
    return in_maps, los, ncol


def run(inputs, trace=False, **kw):
    in_maps, los, ncol = _prep_inputs(
        inputs["s"], inputs["v"], inputs["batch_mask"], inputs["w1"],
        inputs["w2"], inputs["mlp_w1"], inputs["mlp_b1"], inputs["mlp_w2"],
        inputs["mlp_b2"], inputs["out_w"], inputs["out_b"])
    nc = _get_nc(ncol)
    res = run_bass_kernel_spmd(nc, in_maps, list(range(N_CORES)),
                               trace=trace, **kw)
    y = np.zeros(B, np.float64)
    for k in range(N_CORES):
        y[los[k]:los[k] + ncol] += res.results[k]["y"][0].astype(np.float64)
    return y.astype(np.float32).reshape(B, 1), res


def kernel(**inputs):
    y, _ = run(inputs)
    return y
